# revision 19
# baseline (speedup 1.0000x reference)
"""Trainium2 Bass kernel for the contrastive memory-bank loss.

Strategy: data-parallel over pixels. Host-side we drop masked-out pixels
(they contribute nothing), pad to a multiple of 8*128, and shard the
surviving pixels across 8 cores. The small memory bank is replicated.

Per-pixel math (temp=0.5, S=256, eps=1e-12), for pixel p with label i,
half h = 1-wm, D = total - block_sum[i] + eps:
    term_sum(p) = sum_s log(E_s + D) - sum_s log(E_s)
with E_s = exp(cos_s/temp) over the selected half of class i.
Since D ~ 9e3 >> E_s ~ 1, log(E_s + D) = log(D) + E_s/D - O((E_s/D)^2),
so  term_sum = S*log(D) + (sum_s E_s)/D - (sum_s cos_s)/temp
to relative accuracy ~1e-9.  Only per-(class,half) sums of E and of cos
are needed - no per-element logs over the big [P, C*2S] matrix.

Each core returns per-class partial sums (contrib, count); the host
all-reduces the 8 partials and applies the final scalar normalization.

Engine split per core: PE does the [P,F]x[F,M] cosine matmuls (bf16),
ScalarE does batched exp (per-partition 1/(fn*temp) scale), VectorE does
the per-(class,half) sums as a bf16 add-tree (tensor_tensor runs 2x,
tensor_reduce only 1x), GPSIMD does squares/casts, DMA broadcasts the
1/|m| row across partitions (stride-0 read) instead of K=1 matmuls.
"""

import sys

sys.path.insert(0, "/opt/trn_rl_repo")

import numpy as np
import ml_dtypes

import concourse.bass as bass
import concourse.bacc as bacc
import concourse.tile as tile
from concourse import mybir
from concourse import hw_specs as _hw_specs
from concourse.bass_utils import run_bass_kernel_spmd

_orig_gat = _hw_specs.get_activation_tables


def _gat_combined(arch):
    t = dict(_orig_gat(arch))
    if "natural_log_exp_and_others" in t:
        for name in ("exp_and_others", "natural_log", "exp_and_friends"):
            if name in t:
                t[name] = set()
    return t


bacc.get_activation_tables = _gat_combined

F = 256          # feature dim
C = 19           # num classes
S = 256          # half-bank size
TWO_S = 2 * S
M = C * TWO_S    # 9728 memory entries
J = 2 * C        # 38 (class, half) blocks
N_CORES = 8
TEMP = 0.5
EPS = 1e-12

f32 = mybir.dt.float32
bf16 = mybir.dt.bfloat16
AF = mybir.ActivationFunctionType
ALU = mybir.AluOpType
X = mybir.AxisListType.X


def build(P):
    """Build the per-core Bass program for P pixels per core (P % 128 == 0)."""
    T = P // 128
    nc = bacc.Bacc("TRN2", target_bir_lowering=False, debug=False,
                   num_devices=N_CORES)

    feats_d = nc.dram_tensor("feats", [F, P], f32, kind="ExternalInput")
    memT_d = nc.dram_tensor("memT", [F, M], bf16, kind="ExternalInput")
    labf_d = nc.dram_tensor("labf", [128, T], f32, kind="ExternalInput")
    jself_d = nc.dram_tensor("jself", [128, T], f32, kind="ExternalInput")
    mskf_d = nc.dram_tensor("mskf", [128, T], f32, kind="ExternalInput")
    out_d = nc.dram_tensor("out", [2, (P // 128) * C], f32,
                           kind="ExternalOutput")

    with tile.TileContext(nc) as tc:
        with (
            tc.tile_pool(name="const", bufs=1) as const,
            tc.tile_pool(name="persist", bufs=1) as persist,
            tc.tile_pool(name="mem", bufs=1) as mem,
            tc.tile_pool(name="work", bufs=3) as work,
            tc.tile_pool(name="epool", bufs=2) as epool,
        ):
            # ---- constants ----
            iota_i = const.tile([128, J], mybir.dt.int32, tag="iotai")
            nc.gpsimd.iota(iota_i, pattern=[[1, J]], base=0,
                           channel_multiplier=0)
            iota38 = const.tile([128, J], f32, tag="iota38")
            nc.vector.tensor_copy(out=iota38, in_=iota_i)
            ones_col = const.tile([128, 1], f32, tag="ones_col")
            nc.vector.memset(ones_col, 1.0)
            ones_b = const.tile([128, 1], bf16, tag="ones_b")
            nc.vector.memset(ones_b, 1.0)

            # ---- small per-pixel inputs ----
            labf = persist.tile([128, T], f32, tag="labf")
            nc.sync.dma_start(out=labf, in_=labf_d[:, :])
            jself = persist.tile([128, T], f32, tag="jself")
            nc.sync.dma_start(out=jself, in_=jself_d[:, :])
            mskf = persist.tile([128, T], f32, tag="mskf")
            nc.sync.dma_start(out=mskf, in_=mskf_d[:, :])

            # long-lived big tensors
            fb16 = [persist.tile([128, P], bf16, tag=f"fb{k}", name=f"fb{k}")
                    for k in range(2)]
            mn_k = [mem.tile([128, M], bf16, tag=f"mn{k}", name=f"mn{k}")
                    for k in range(2)]
            s_tiles = persist.tile([128, T], f32, tag="stl")
            hcos = persist.tile([128, T * J], f32, tag="hcos")

            def add_tree(src, out_f32):
                """Per-block free-dim sums: [128, nj, 256] bf16 -> [128, nj]
                f32 via in-place halving adds (tensor_tensor runs 2x mode;
                tensor_reduce is 1x-only) and a small 1x reduce tail."""
                w = S
                while w > 16:
                    w //= 2
                    nc.vector.tensor_add(out=src[:, :, 0:w],
                                         in0=src[:, :, 0:w],
                                         in1=src[:, :, w:2 * w])
                nc.vector.tensor_reduce(out=out_f32, in_=src[:, :, 0:16],
                                        axis=X, op=ALU.add)

            # ================= PREP (scoped; freed before main) ========
            # Per-class pipeline: each 512-wide class chunk flows through
            # square -> ones-matmul -> 1/sqrt -> partition-broadcast ->
            # normalize independently, so the main loop (which reads mn_k
            # per class slice) can start as soon as early classes land.
            with (
                tc.tile_pool(name="prep", bufs=2) as prep,
                tc.tile_pool(name="mraw_p", bufs=1) as mraw_p,
                tc.tile_pool(name="rows", bufs=1) as rows,
                tc.tile_pool(name="dram", bufs=4, space="DRAM") as dram,
                tc.tile_pool(name="pp", bufs=2, space="PSUM") as pp,
            ):
                # ---- feats: load, bf16 cast, per-pixel 1/(fn*temp) ----
                fn_row = prep.tile([1, P], f32, tag="fnrow", bufs=1)
                for k in range(2):
                    fk = prep.tile([128, P], f32, tag=f"f{k}", bufs=1)
                    nc.sync.dma_start(out=fk,
                                      in_=feats_d[k * 128:(k + 1) * 128, :])
                    nc.vector.tensor_copy(out=fb16[k], in_=fk)
                    sq = prep.tile([128, P], bf16, tag=f"fsq{k}", bufs=1)
                    nc.vector.tensor_mul(out=sq, in0=fk, in1=fk)
                    fsq = sq if k == 0 else fsq
                    if k == 1:
                        for off in range(0, P, 512):
                            n = min(512, P - off)
                            pfn = pp.tile([1, 512], f32, tag="pp")
                            nc.tensor.matmul(pfn[:, :n], ones_b,
                                             fsq[:, off:off + n],
                                             start=True, stop=False)
                            nc.tensor.matmul(pfn[:, :n], ones_b,
                                             sq[:, off:off + n],
                                             start=False, stop=True)
                            nc.scalar.copy(out=fn_row[:, off:off + n],
                                           in_=pfn[:, :n])
                d_fn = dram.tile([1, P], f32, tag="dfn")
                nc.sync.dma_start(out=d_fn, in_=fn_row[0:1, :])
                s_pre = persist.tile([128, T], f32, tag="spre")
                nc.sync.dma_start(
                    out=s_pre, in_=d_fn.rearrange("o (t p) -> (o p) t", p=128))
                r1 = prep.tile([128, T], f32, tag="r1")
                nc.vector.reciprocal(out=r1, in_=s_pre)
                # sqrt((1/temp^2) * (1/fn^2)) = 1/(fn*temp)
                nc.scalar.activation(out=s_tiles, in_=r1, func=AF.Sqrt,
                                     scale=1.0 / (TEMP * TEMP))

                # ---- memory bank: chunked load + per-class normalize ----
                mraw = []
                for k in range(2):
                    mr = mraw_p.tile([128, M], bf16, tag=f"mraw{k}",
                                     name=f"mraw{k}")
                    for g in range(4):
                        lo = g * (M // 4)
                        hi = M if g == 3 else (g + 1) * (M // 4)
                        nc.sync.dma_start(
                            out=mr[:, lo:hi],
                            in_=memT_d[k * 128:(k + 1) * 128, lo:hi])
                    mraw.append(mr)
                bcast = rows.tile([128, M], bf16, tag="bcast")
                for ci in range(C):
                    sl = slice(ci * 512, ci * 512 + 512)
                    pmn = pp.tile([1, 512], f32, tag="pp")
                    for k in range(2):
                        sqm = prep.tile([128, 512], bf16, tag="sqm", bufs=4)
                        nc.vector.tensor_mul(out=sqm, in0=mraw[k][:, sl],
                                             in1=mraw[k][:, sl])
                        nc.tensor.matmul(pmn, ones_b, sqm,
                                         start=(k == 0), stop=(k == 1))
                    stg = prep.tile([1, 512], f32, tag="stg", bufs=4)
                    nc.vector.tensor_copy(out=stg, in_=pmn)
                    d_ch = dram.tile([1, 512], f32, tag="dch", bufs=4)
                    nc.sync.dma_start(out=d_ch, in_=stg[0:1, :])
                    rt4 = prep.tile([128, 4], f32, tag="rt4", bufs=4)
                    nc.sync.dma_start(
                        out=rt4,
                        in_=d_ch.rearrange("o (q p) -> (o p) q", p=128))
                    nc.vector.reciprocal(out=rt4, in_=rt4)
                    nc.scalar.activation(out=rt4, in_=rt4, func=AF.Sqrt)
                    rb4 = prep.tile([128, 4], bf16, tag="rb4", bufs=4)
                    nc.vector.tensor_copy(out=rb4, in_=rt4)
                    d_bc = dram.tile([1, 512], bf16, tag="dbc", bufs=4)
                    nc.sync.dma_start(
                        out=d_bc.rearrange("o (q p) -> (o p) q", p=128),
                        in_=rb4)
                    nc.sync.dma_start(
                        out=bcast[:, sl],
                        in_=d_bc.partition_broadcast(128))
                    for k in range(2):
                        nc.vector.tensor_mul(out=mn_k[k][:, sl],
                                             in0=mraw[k][:, sl],
                                             in1=bcast[:, sl])

            # ================= end PREP ================================

            # ---- per-tile result columns (batched tail after loop) ----
            hsum_all = persist.tile([128, T, J], f32, tag="hsum_all")
            ohm_all = persist.tile([128, T, C], f32, tag="ohm_all")
            oht_all = persist.tile([128, T, C], f32, tag="oht_all")
            total_all = persist.tile([128, T], f32, tag="total_all")
            ownb_all = persist.tile([128, T], f32, tag="ownb_all")
            pos1_all = persist.tile([128, T], f32, tag="pos1_all")
            poscos_all = persist.tile([128, T], f32, tag="poscos_all")

            # ---- main loop over pixel tiles: pure mm -> exp -> tree ----
            batches = [(0, 3), (3, 3), (6, 3), (9, 3), (12, 3), (15, 2),
                       (17, 2)]
            with tc.tile_pool(name="psum_mm", bufs=2, space="PSUM") as psum_mm:
                for t in range(T):
                    ts = slice(t * 128, (t + 1) * 128)
                    s_col = s_tiles[:, t:t + 1]
                    E = epool.tile([128, J, S], bf16, tag="E")
                    for c0, nb in batches:
                        ps = psum_mm.tile([128, 3 * 512], f32, tag="mm")
                        for k in range(2):
                            for i in range(nb):
                                c = c0 + i
                                nc.tensor.matmul(
                                    ps[:, i * 512:(i + 1) * 512],
                                    fb16[k][:, ts],
                                    mn_k[k][:, c * 512:(c + 1) * 512],
                                    start=(k == 0), stop=(k == 1))
                        nc.scalar.activation(
                            out=E[:, 2 * c0:2 * (c0 + nb), :],
                            in_=ps[:, :nb * 512], func=AF.Exp, scale=s_col)
                    add_tree(E, hsum_all[:, t, :])

                # ---- post loop: hv / hcos (off the critical path) ----
                hv = []
                for k in range(2):
                    hvf = work.tile([128, J], f32, tag=f"hvf{k}",
                                    name=f"hvf{k}")
                    nc.vector.tensor_reduce(
                        out=hvf,
                        in_=mn_k[k].rearrange("p (j s) -> p j s", s=S),
                        axis=X, op=ALU.add)
                    hvb = work.tile([128, J], bf16, tag=f"hv{k}",
                                    name=f"hv{k}")
                    nc.vector.tensor_copy(out=hvb, in_=hvf)
                    hv.append(hvb)
                for t in range(T):
                    phc = psum_mm.tile([128, J], f32, tag="hc")
                    for k in range(2):
                        nc.tensor.matmul(
                            phc, fb16[k][:, t * 128:(t + 1) * 128],
                            hv[k], start=(k == 0), stop=(k == 1))
                    nc.scalar.copy(out=hcos[:, t * J:(t + 1) * J], in_=phc)

            # ---- batched block sums and per-tile selections ----
            h4 = hsum_all.rearrange("p t (c h) -> p t c h", h=2)
            bsum_all = work.tile([128, T, C], f32, tag="bsum_all")
            nc.vector.tensor_add(out=bsum_all, in0=h4[:, :, :, 0],
                                 in1=h4[:, :, :, 1])
            nc.vector.tensor_reduce(out=total_all, in_=bsum_all, axis=X,
                                    op=ALU.add)
            for t in range(T):
                j19 = work.tile([128, C], f32, tag="j19")
                nc.vector.scalar_tensor_tensor(
                    out=j19, in0=iota38[:, :C], scalar=labf[:, t:t + 1],
                    in1=bsum_all[:, t, :], op0=ALU.is_equal, op1=ALU.mult,
                    accum_out=ownb_all[:, t:t + 1])
                j38 = work.tile([128, J], f32, tag="j38")
                nc.vector.scalar_tensor_tensor(
                    out=j38, in0=iota38, scalar=jself[:, t:t + 1],
                    in1=hsum_all[:, t, :], op0=ALU.is_equal, op1=ALU.mult,
                    accum_out=pos1_all[:, t:t + 1])
                j38b = work.tile([128, J], f32, tag="j38b")
                nc.vector.scalar_tensor_tensor(
                    out=j38b, in0=iota38, scalar=jself[:, t:t + 1],
                    in1=hcos[:, t * J:(t + 1) * J], op0=ALU.is_equal,
                    op1=ALU.mult, accum_out=poscos_all[:, t:t + 1])
                nc.vector.tensor_scalar(
                    out=ohm_all[:, t, :], in0=iota38[:, :C],
                    scalar1=labf[:, t:t + 1], scalar2=mskf[:, t:t + 1],
                    op0=ALU.is_equal, op1=ALU.mult)

            # ---- batched per-pixel tail over all T columns ----
            D_all = work.tile([128, T], f32, tag="D_all")
            nc.vector.scalar_tensor_tensor(
                out=D_all, in0=total_all, scalar=float(EPS), in1=ownb_all,
                op0=ALU.add, op1=ALU.subtract)
            rD = work.tile([128, T], f32, tag="rD")
            nc.vector.reciprocal(out=rD, in_=D_all)
            lnD = work.tile([128, T], f32, tag="lnD")
            nc.scalar.activation(out=lnD, in_=D_all, func=AF.Ln)
            ta = work.tile([128, T], f32, tag="ta")
            nc.vector.tensor_mul(out=ta, in0=pos1_all, in1=rD)
            tb = work.tile([128, T], f32, tag="tb")
            nc.vector.scalar_tensor_tensor(
                out=tb, in0=lnD, scalar=float(S), in1=ta,
                op0=ALU.mult, op1=ALU.add)
            tcm = work.tile([128, T], f32, tag="tcm")
            nc.vector.tensor_mul(out=tcm, in0=poscos_all, in1=s_tiles)
            term_all = work.tile([128, T], f32, tag="term_all")
            nc.vector.tensor_sub(out=term_all, in0=tb, in1=tcm)
            term_bc = bass.AP(tensor=term_all.tensor, offset=term_all.offset,
                              ap=[*term_all.ap, [0, C]])
            nc.vector.tensor_mul(out=oht_all, in0=ohm_all, in1=term_bc)

            # ---- finalize: partition-reduce [128, T*C] -> [1, T*C] ----
            TC = T * C
            stage = persist.tile([1, 2 * TC], f32, tag="stage")
            oht_fl = oht_all.rearrange("p t c -> p (t c)")
            ohm_fl = ohm_all.rearrange("p t c -> p (t c)")
            with tc.tile_pool(name="psum_out", bufs=2, space="PSUM") as psum_o:
                po = psum_o.tile([1, TC], f32, tag="po")
                nc.tensor.matmul(po, ones_col, oht_fl, start=True, stop=True)
                nc.scalar.copy(out=stage[0:1, :TC], in_=po)
                po2 = psum_o.tile([1, TC], f32, tag="po2")
                nc.tensor.matmul(po2, ones_col, ohm_fl, start=True, stop=True)
                nc.scalar.copy(out=stage[0:1, TC:], in_=po2)
            nc.sync.dma_start(out=out_d.rearrange("a b -> (a b)")[None, :],
                              in_=stage)

    nc.finalize()
    return nc


_CACHE = {}


def get_program(P):
    if P not in _CACHE:
        _CACHE[P] = build(P)
    return _CACHE[P]


def prepare_inputs(memory_bank, pred_rep, labels, mask, which_memory):
    """Host-side sharding: compact masked pixels, pad, split across cores."""
    memory_bank = np.asarray(memory_bank, dtype=np.float32)
    pred_rep = np.asarray(pred_rep, dtype=np.float32)
    lab = np.asarray(labels).reshape(-1).astype(np.int64)
    msk = np.asarray(mask).reshape(-1).astype(bool)
    wm = np.asarray(which_memory).reshape(-1).astype(np.int64)

    memT = np.ascontiguousarray(
        memory_bank.reshape(M, F).T).astype(ml_dtypes.bfloat16)

    featsT = np.ascontiguousarray(
        pred_rep.transpose(1, 0, 2, 3).reshape(F, -1))

    sel = np.flatnonzero(msk)
    n_sel = len(sel)
    unit = N_CORES * 128
    P_tot = max(((n_sel + unit - 1) // unit) * unit, unit)
    P = P_tot // N_CORES
    T = P // 128

    f_pad = np.ones((F, P_tot), np.float32)
    f_pad[:, :n_sel] = featsT[:, sel]
    lab_pad = np.zeros(P_tot, np.float32)
    lab_pad[:n_sel] = lab[sel]
    jsel_pad = np.zeros(P_tot, np.float32)
    jsel_pad[:n_sel] = 2 * lab[sel] + (1 - wm[sel])
    msk_pad = np.zeros(P_tot, np.float32)
    msk_pad[:n_sel] = 1.0

    in_maps = []
    for i in range(N_CORES):
        cs = slice(i * P, (i + 1) * P)
        in_maps.append({
            "feats": np.ascontiguousarray(f_pad[:, cs]),
            "memT": memT,
            "labf": np.ascontiguousarray(lab_pad[cs].reshape(T, 128).T),
            "jself": np.ascontiguousarray(jsel_pad[cs].reshape(T, 128).T),
            "mskf": np.ascontiguousarray(msk_pad[cs].reshape(T, 128).T),
        })
    return P, in_maps


def finalize(outs, num_classes):
    agg = np.zeros((2, C), np.float64)
    for o in outs:
        a = np.asarray(o, dtype=np.float64)
        agg += a.reshape(2, -1, C).sum(axis=1)
    contrib, cnt = agg[0], agg[1]
    nz = cnt > 0.5
    per_class = np.where(nz, contrib / (np.maximum(cnt, 1.0) * S), 0.0)
    loss = per_class[:num_classes].sum() / max(int(nz[:num_classes].sum()), 1)
    return np.float32(loss)


def kernel(memory_bank, pred_rep, labels, mask, which_memory, num_classes,
           temp=0.5):
    assert int(num_classes) == C and abs(temp - TEMP) < 1e-12
    P, in_maps = prepare_inputs(memory_bank, pred_rep, labels, mask,
                                which_memory)
    nc = get_program(P)
    res = run_bass_kernel_spmd(nc, in_maps, core_ids=list(range(N_CORES)))
    outs = [res.results[i]["out"] for i in range(N_CORES)]
    return finalize(outs, int(num_classes))


# revision 20
# speedup vs baseline: 2.1591x; 2.1591x over previous
"""Trainium2 Bass kernel for the contrastive memory-bank loss.

Strategy: data-parallel over pixels. Host-side we drop masked-out pixels
(they contribute nothing), pad to a multiple of 8*128, and shard the
surviving pixels across 8 cores. The small memory bank is replicated.

Per-pixel math (temp=0.5, S=256, eps=1e-12), for pixel p with label i,
half h = 1-wm, D = total - block_sum[i] + eps:
    term_sum(p) = sum_s log(E_s + D) - sum_s log(E_s)
with E_s = exp(cos_s/temp) over the selected half of class i.
Since D ~ 9e3 >> E_s ~ 1, log(E_s + D) = log(D) + E_s/D - O((E_s/D)^2),
so  term_sum = S*log(D) + (sum_s E_s)/D - (sum_s cos_s)/temp
to relative accuracy ~1e-9.  Only per-(class,half) sums of E and of cos
are needed - no per-element logs over the big [P, C*2S] matrix.

Each core returns per-class partial sums (contrib, count); the host
all-reduces the 8 partials and applies the final scalar normalization.

Engine split per core: PE does the [P,F]x[F,M] cosine matmuls (bf16),
ScalarE does batched exp (per-partition 1/(fn*temp) scale), VectorE does
the per-(class,half) sums as a bf16 add-tree (tensor_tensor runs 2x,
tensor_reduce only 1x), GPSIMD does squares/casts, DMA broadcasts the
1/|m| row across partitions (stride-0 read) instead of K=1 matmuls.
"""

import sys

sys.path.insert(0, "/opt/trn_rl_repo")

import numpy as np
import ml_dtypes

import concourse.bass as bass
import concourse.bacc as bacc
import concourse.tile as tile
from concourse import mybir
from concourse import hw_specs as _hw_specs
from concourse.bass_utils import run_bass_kernel_spmd

_orig_gat = _hw_specs.get_activation_tables


def _gat_combined(arch):
    t = dict(_orig_gat(arch))
    if "natural_log_exp_and_others" in t:
        for name in ("exp_and_others", "natural_log", "exp_and_friends"):
            if name in t:
                t[name] = set()
    return t


bacc.get_activation_tables = _gat_combined

F = 256          # feature dim
C = 19           # num classes
S = 256          # half-bank size
TWO_S = 2 * S
M = C * TWO_S    # 9728 memory entries
J = 2 * C        # 38 (class, half) blocks
N_CORES = 8
TEMP = 0.5
EPS = 1e-12

f32 = mybir.dt.float32
bf16 = mybir.dt.bfloat16
AF = mybir.ActivationFunctionType
ALU = mybir.AluOpType
X = mybir.AxisListType.X


def build(P):
    """Build the per-core Bass program for P pixels per core (P % 128 == 0)."""
    T = P // 128
    nc = bacc.Bacc("TRN2", target_bir_lowering=False, debug=False,
                   num_devices=N_CORES)

    feats_d = nc.dram_tensor("feats", [F, P], f32, kind="ExternalInput")
    memT_d = nc.dram_tensor("memT", [F, M], bf16, kind="ExternalInput")
    labf_d = nc.dram_tensor("labf", [128, T], f32, kind="ExternalInput")
    jself_d = nc.dram_tensor("jself", [128, T], f32, kind="ExternalInput")
    mskf_d = nc.dram_tensor("mskf", [128, T], f32, kind="ExternalInput")
    out_d = nc.dram_tensor("out", [2, (P // 128) * C], f32,
                           kind="ExternalOutput")

    with tile.TileContext(nc) as tc:
        with (
            tc.tile_pool(name="const", bufs=1) as const,
            tc.tile_pool(name="persist", bufs=1) as persist,
            tc.tile_pool(name="mem", bufs=1) as mem,
            tc.tile_pool(name="work", bufs=3) as work,
            tc.tile_pool(name="epool", bufs=2) as epool,
        ):
            # ---- constants ----
            iota_i = const.tile([128, J], mybir.dt.int32, tag="iotai")
            nc.gpsimd.iota(iota_i, pattern=[[1, J]], base=0,
                           channel_multiplier=0)
            iota38 = const.tile([128, J], f32, tag="iota38")
            nc.vector.tensor_copy(out=iota38, in_=iota_i)
            ones_col = const.tile([128, 1], f32, tag="ones_col")
            nc.vector.memset(ones_col, 1.0)
            ones_b = const.tile([128, 1], bf16, tag="ones_b")
            nc.vector.memset(ones_b, 1.0)

            # ---- small per-pixel inputs ----
            labf = persist.tile([128, T], f32, tag="labf")
            nc.sync.dma_start(out=labf, in_=labf_d[:, :])
            jself = persist.tile([128, T], f32, tag="jself")
            nc.sync.dma_start(out=jself, in_=jself_d[:, :])
            mskf = persist.tile([128, T], f32, tag="mskf")
            nc.sync.dma_start(out=mskf, in_=mskf_d[:, :])

            # long-lived big tensors
            fb16 = [persist.tile([128, P], bf16, tag=f"fb{k}", name=f"fb{k}")
                    for k in range(2)]
            mn_k = [mem.tile([128, M], bf16, tag=f"mn{k}", name=f"mn{k}")
                    for k in range(2)]
            s_tiles = persist.tile([128, T], f32, tag="stl")
            hcos = persist.tile([128, T * J], f32, tag="hcos")

            def add_tree(src, out_f32):
                """Per-block free-dim sums: [128, nj, 256] bf16 -> [128, nj]
                f32 via in-place halving adds (tensor_tensor runs 2x mode;
                tensor_reduce is 1x-only) and a small 1x reduce tail."""
                w = S
                while w > 16:
                    w //= 2
                    nc.vector.tensor_add(out=src[:, :, 0:w],
                                         in0=src[:, :, 0:w],
                                         in1=src[:, :, w:2 * w])
                nc.vector.tensor_reduce(out=out_f32, in_=src[:, :, 0:16],
                                        axis=X, op=ALU.add)

            # ================= PREP (scoped; freed before main) ========
            # Per-class pipeline: each 512-wide class chunk flows through
            # square -> ones-matmul -> 1/sqrt -> partition-broadcast ->
            # normalize independently, so the main loop (which reads mn_k
            # per class slice) can start as soon as early classes land.
            with (
                tc.tile_pool(name="prep", bufs=2) as prep,
                tc.tile_pool(name="mraw_p", bufs=1) as mraw_p,
                tc.tile_pool(name="rows", bufs=1) as rows,
                tc.tile_pool(name="dram", bufs=4, space="DRAM") as dram,
                tc.tile_pool(name="pp", bufs=2, space="PSUM") as pp,
            ):
                # ---- feats: load, bf16 cast, per-pixel 1/(fn*temp) ----
                fn_row = prep.tile([1, P], f32, tag="fnrow", bufs=1)
                for k in range(2):
                    fk = prep.tile([128, P], f32, tag=f"f{k}", bufs=1)
                    nc.sync.dma_start(out=fk,
                                      in_=feats_d[k * 128:(k + 1) * 128, :])
                    nc.vector.tensor_copy(out=fb16[k], in_=fk)
                    sq = prep.tile([128, P], bf16, tag=f"fsq{k}", bufs=1)
                    nc.vector.tensor_mul(out=sq, in0=fk, in1=fk)
                    fsq = sq if k == 0 else fsq
                    if k == 1:
                        for off in range(0, P, 512):
                            n = min(512, P - off)
                            pfn = pp.tile([1, 512], f32, tag="pp")
                            nc.tensor.matmul(pfn[:, :n], ones_b,
                                             fsq[:, off:off + n],
                                             start=True, stop=False)
                            nc.tensor.matmul(pfn[:, :n], ones_b,
                                             sq[:, off:off + n],
                                             start=False, stop=True)
                            nc.scalar.copy(out=fn_row[:, off:off + n],
                                           in_=pfn[:, :n])
                d_fn = dram.tile([1, P], f32, tag="dfn")
                nc.sync.dma_start(out=d_fn, in_=fn_row[0:1, :])
                s_pre = persist.tile([128, T], f32, tag="spre")
                nc.sync.dma_start(
                    out=s_pre, in_=d_fn.rearrange("o (t p) -> (o p) t", p=128))
                r1 = prep.tile([128, T], f32, tag="r1")
                nc.vector.reciprocal(out=r1, in_=s_pre)
                # sqrt((1/temp^2) * (1/fn^2)) = 1/(fn*temp)
                nc.scalar.activation(out=s_tiles, in_=r1, func=AF.Sqrt,
                                     scale=1.0 / (TEMP * TEMP))

                # ---- memory bank: stage-major normalize pipeline ----
                # Stage 1 (per class): square -> ones-matmul -> psum row out.
                # Stages 2-4 (global): 1/sqrt on [128,76], broadcast via
                # stride-0 DMA, two big normalize multiplies. Stage-major
                # emission keeps the in-order DVE stream free of per-class
                # DMA-latency stalls.
                mraw = []
                for k in range(2):
                    mr = mraw_p.tile([128, M], bf16, tag=f"mraw{k}",
                                     name=f"mraw{k}")
                    for g in range(4):
                        lo = g * (M // 4)
                        hi = M if g == 3 else (g + 1) * (M // 4)
                        nc.sync.dma_start(
                            out=mr[:, lo:hi],
                            in_=memT_d[k * 128:(k + 1) * 128, lo:hi])
                    mraw.append(mr)
                rinv_t = prep.tile([128, M // 128], f32, tag="rinvt")
                for ci in range(C):
                    sl = slice(ci * 512, ci * 512 + 512)
                    pmn = pp.tile([1, 512], f32, tag="pp")
                    for k in range(2):
                        sqm = prep.tile([128, 512], bf16, tag="sqm", bufs=4)
                        nc.vector.tensor_mul(out=sqm, in0=mraw[k][:, sl],
                                             in1=mraw[k][:, sl])
                        nc.tensor.matmul(pmn, ones_b, sqm,
                                         start=(k == 0), stop=(k == 1))
                    stg = prep.tile([1, 512], f32, tag="stg", bufs=4)
                    nc.vector.tensor_copy(out=stg, in_=pmn)
                    d_ch = dram.tile([1, 512], f32, tag="dch", bufs=8)
                    nc.sync.dma_start(out=d_ch, in_=stg[0:1, :])
                    nc.sync.dma_start(
                        out=rinv_t[:, ci * 4:(ci + 1) * 4],
                        in_=d_ch.rearrange("o (q p) -> (o p) q", p=128))
                nc.vector.reciprocal(out=rinv_t, in_=rinv_t)
                nc.scalar.activation(out=rinv_t, in_=rinv_t, func=AF.Sqrt)
                rinv_tb = prep.tile([128, M // 128], bf16, tag="rinvtb")
                nc.vector.tensor_copy(out=rinv_tb, in_=rinv_t)
                d_rv = dram.tile([1, M], bf16, tag="drv")
                nc.sync.dma_start(
                    out=d_rv.rearrange("o (c p) -> (o p) c", p=128),
                    in_=rinv_tb)
                bcast = rows.tile([128, M], bf16, tag="bcast")
                nc.sync.dma_start(out=bcast,
                                  in_=d_rv.partition_broadcast(128))
                for k in range(2):
                    nc.vector.tensor_mul(out=mn_k[k], in0=mraw[k], in1=bcast)
            # ================= end PREP ================================

            # ---- per-tile result columns (batched tail after loop) ----
            hsum_all = persist.tile([128, T, J], f32, tag="hsum_all")
            ohm_all = persist.tile([128, T, C], f32, tag="ohm_all")
            oht_all = persist.tile([128, T, C], f32, tag="oht_all")
            total_all = persist.tile([128, T], f32, tag="total_all")
            ownb_all = persist.tile([128, T], f32, tag="ownb_all")
            pos1_all = persist.tile([128, T], f32, tag="pos1_all")
            poscos_all = persist.tile([128, T], f32, tag="poscos_all")

            # ---- main loop over pixel tiles: pure mm -> exp -> tree ----
            batches = [(0, 3), (3, 3), (6, 3), (9, 3), (12, 3), (15, 2),
                       (17, 2)]
            with tc.tile_pool(name="psum_mm", bufs=2, space="PSUM") as psum_mm:
                for t in range(T):
                    ts = slice(t * 128, (t + 1) * 128)
                    s_col = s_tiles[:, t:t + 1]
                    E = epool.tile([128, J, S], bf16, tag="E")
                    for c0, nb in batches:
                        ps = psum_mm.tile([128, 3 * 512], f32, tag="mm")
                        for k in range(2):
                            for i in range(nb):
                                c = c0 + i
                                nc.tensor.matmul(
                                    ps[:, i * 512:(i + 1) * 512],
                                    fb16[k][:, ts],
                                    mn_k[k][:, c * 512:(c + 1) * 512],
                                    start=(k == 0), stop=(k == 1))
                        nc.scalar.activation(
                            out=E[:, 2 * c0:2 * (c0 + nb), :],
                            in_=ps[:, :nb * 512], func=AF.Exp, scale=s_col)
                    add_tree(E, hsum_all[:, t, :])

                # ---- post loop: hv / hcos (off the critical path) ----
                hv = []
                for k in range(2):
                    hvf = work.tile([128, J], f32, tag=f"hvf{k}",
                                    name=f"hvf{k}")
                    nc.vector.tensor_reduce(
                        out=hvf,
                        in_=mn_k[k].rearrange("p (j s) -> p j s", s=S),
                        axis=X, op=ALU.add)
                    hvb = work.tile([128, J], bf16, tag=f"hv{k}",
                                    name=f"hv{k}")
                    nc.vector.tensor_copy(out=hvb, in_=hvf)
                    hv.append(hvb)
                for t in range(T):
                    phc = psum_mm.tile([128, J], f32, tag="hc")
                    for k in range(2):
                        nc.tensor.matmul(
                            phc, fb16[k][:, t * 128:(t + 1) * 128],
                            hv[k], start=(k == 0), stop=(k == 1))
                    nc.scalar.copy(out=hcos[:, t * J:(t + 1) * J], in_=phc)

            # ---- batched block sums and per-tile selections ----
            h4 = hsum_all.rearrange("p t (c h) -> p t c h", h=2)
            bsum_all = work.tile([128, T, C], f32, tag="bsum_all")
            nc.vector.tensor_add(out=bsum_all, in0=h4[:, :, :, 0],
                                 in1=h4[:, :, :, 1])
            nc.vector.tensor_reduce(out=total_all, in_=bsum_all, axis=X,
                                    op=ALU.add)
            for t in range(T):
                j19 = work.tile([128, C], f32, tag="j19")
                nc.vector.scalar_tensor_tensor(
                    out=j19, in0=iota38[:, :C], scalar=labf[:, t:t + 1],
                    in1=bsum_all[:, t, :], op0=ALU.is_equal, op1=ALU.mult,
                    accum_out=ownb_all[:, t:t + 1])
                j38 = work.tile([128, J], f32, tag="j38")
                nc.vector.scalar_tensor_tensor(
                    out=j38, in0=iota38, scalar=jself[:, t:t + 1],
                    in1=hsum_all[:, t, :], op0=ALU.is_equal, op1=ALU.mult,
                    accum_out=pos1_all[:, t:t + 1])
                j38b = work.tile([128, J], f32, tag="j38b")
                nc.vector.scalar_tensor_tensor(
                    out=j38b, in0=iota38, scalar=jself[:, t:t + 1],
                    in1=hcos[:, t * J:(t + 1) * J], op0=ALU.is_equal,
                    op1=ALU.mult, accum_out=poscos_all[:, t:t + 1])
                nc.vector.tensor_scalar(
                    out=ohm_all[:, t, :], in0=iota38[:, :C],
                    scalar1=labf[:, t:t + 1], scalar2=mskf[:, t:t + 1],
                    op0=ALU.is_equal, op1=ALU.mult)

            # ---- batched per-pixel tail over all T columns ----
            D_all = work.tile([128, T], f32, tag="D_all")
            nc.vector.scalar_tensor_tensor(
                out=D_all, in0=total_all, scalar=float(EPS), in1=ownb_all,
                op0=ALU.add, op1=ALU.subtract)
            rD = work.tile([128, T], f32, tag="rD")
            nc.vector.reciprocal(out=rD, in_=D_all)
            lnD = work.tile([128, T], f32, tag="lnD")
            nc.scalar.activation(out=lnD, in_=D_all, func=AF.Ln)
            ta = work.tile([128, T], f32, tag="ta")
            nc.vector.tensor_mul(out=ta, in0=pos1_all, in1=rD)
            tb = work.tile([128, T], f32, tag="tb")
            nc.vector.scalar_tensor_tensor(
                out=tb, in0=lnD, scalar=float(S), in1=ta,
                op0=ALU.mult, op1=ALU.add)
            tcm = work.tile([128, T], f32, tag="tcm")
            nc.vector.tensor_mul(out=tcm, in0=poscos_all, in1=s_tiles)
            term_all = work.tile([128, T], f32, tag="term_all")
            nc.vector.tensor_sub(out=term_all, in0=tb, in1=tcm)
            term_bc = bass.AP(tensor=term_all.tensor, offset=term_all.offset,
                              ap=[*term_all.ap, [0, C]])
            nc.vector.tensor_mul(out=oht_all, in0=ohm_all, in1=term_bc)

            # ---- finalize: partition-reduce [128, T*C] -> [1, T*C] ----
            TC = T * C
            stage = persist.tile([1, 2 * TC], f32, tag="stage")
            oht_fl = oht_all.rearrange("p t c -> p (t c)")
            ohm_fl = ohm_all.rearrange("p t c -> p (t c)")
            with tc.tile_pool(name="psum_out", bufs=2, space="PSUM") as psum_o:
                po = psum_o.tile([1, TC], f32, tag="po")
                nc.tensor.matmul(po, ones_col, oht_fl, start=True, stop=True)
                nc.scalar.copy(out=stage[0:1, :TC], in_=po)
                po2 = psum_o.tile([1, TC], f32, tag="po2")
                nc.tensor.matmul(po2, ones_col, ohm_fl, start=True, stop=True)
                nc.scalar.copy(out=stage[0:1, TC:], in_=po2)
            nc.sync.dma_start(out=out_d.rearrange("a b -> (a b)")[None, :],
                              in_=stage)

    nc.finalize()
    return nc


_CACHE = {}


def get_program(P):
    if P not in _CACHE:
        _CACHE[P] = build(P)
    return _CACHE[P]


def prepare_inputs(memory_bank, pred_rep, labels, mask, which_memory):
    """Host-side sharding: compact masked pixels, pad, split across cores."""
    memory_bank = np.asarray(memory_bank, dtype=np.float32)
    pred_rep = np.asarray(pred_rep, dtype=np.float32)
    lab = np.asarray(labels).reshape(-1).astype(np.int64)
    msk = np.asarray(mask).reshape(-1).astype(bool)
    wm = np.asarray(which_memory).reshape(-1).astype(np.int64)

    memT = np.ascontiguousarray(
        memory_bank.reshape(M, F).T).astype(ml_dtypes.bfloat16)

    featsT = np.ascontiguousarray(
        pred_rep.transpose(1, 0, 2, 3).reshape(F, -1))

    sel = np.flatnonzero(msk)
    n_sel = len(sel)
    unit = N_CORES * 128
    P_tot = max(((n_sel + unit - 1) // unit) * unit, unit)
    P = P_tot // N_CORES
    T = P // 128

    f_pad = np.ones((F, P_tot), np.float32)
    f_pad[:, :n_sel] = featsT[:, sel]
    lab_pad = np.zeros(P_tot, np.float32)
    lab_pad[:n_sel] = lab[sel]
    jsel_pad = np.zeros(P_tot, np.float32)
    jsel_pad[:n_sel] = 2 * lab[sel] + (1 - wm[sel])
    msk_pad = np.zeros(P_tot, np.float32)
    msk_pad[:n_sel] = 1.0

    in_maps = []
    for i in range(N_CORES):
        cs = slice(i * P, (i + 1) * P)
        in_maps.append({
            "feats": np.ascontiguousarray(f_pad[:, cs]),
            "memT": memT,
            "labf": np.ascontiguousarray(lab_pad[cs].reshape(T, 128).T),
            "jself": np.ascontiguousarray(jsel_pad[cs].reshape(T, 128).T),
            "mskf": np.ascontiguousarray(msk_pad[cs].reshape(T, 128).T),
        })
    return P, in_maps


def finalize(outs, num_classes):
    agg = np.zeros((2, C), np.float64)
    for o in outs:
        a = np.asarray(o, dtype=np.float64)
        agg += a.reshape(2, -1, C).sum(axis=1)
    contrib, cnt = agg[0], agg[1]
    nz = cnt > 0.5
    per_class = np.where(nz, contrib / (np.maximum(cnt, 1.0) * S), 0.0)
    loss = per_class[:num_classes].sum() / max(int(nz[:num_classes].sum()), 1)
    return np.float32(loss)


def kernel(memory_bank, pred_rep, labels, mask, which_memory, num_classes,
           temp=0.5):
    assert int(num_classes) == C and abs(temp - TEMP) < 1e-12
    P, in_maps = prepare_inputs(memory_bank, pred_rep, labels, mask,
                                which_memory)
    nc = get_program(P)
    res = run_bass_kernel_spmd(nc, in_maps, core_ids=list(range(N_CORES)))
    outs = [res.results[i]["out"] for i in range(N_CORES)]
    return finalize(outs, int(num_classes))
